# revision 1
# baseline (speedup 1.0000x reference)
"""Multi-head causal attention (B=2, T=2048, D=1024, H=16) on 8 TRN2 cores.

Sharding: core c handles batch b=c//4 and head group g=c%4 (4 heads each).
Device kernel per core (fp32r matmuls, fp16 attention weights):
  QT/KT = (X Wq/Wk)^T in [head*64, T] layout
  V     = X Wv in [T, 256] layout fp16 with a ones-column per head (V_aug)
  per q-tile (512 q), per head pair, per key-block (128 k, causal):
    ST[k,q] = KT-vs-QT matmul (PSUM, 2 heads row-packed on the PE array)
    PT = exp(ST/8) fp16 (no max subtraction; S/8 ~ N(0,1))
    diag blocks: zero PT where k > q (DVE affine_select)
    CT[65,512] += V_aug-vs-PT matmul (row 64 = softmax denominators l)
  CT_norm = CT[0:64] * (1/l broadcast via gpsimd partition_broadcast)
  OT[d, q-tile] = WO-vs-CT_norm matmul -> fp16 partial, DMA'd per q-tile
Host: out[b] = sum over the 4 head-group cores of OT^T, + bo.

The qt-outer loop pipelines projection slices (n=qt+1) and output
projection (n=qt) into the ACT-bound attention phase.
"""

import numpy as np

B, T, D, H, HD = 2, 2048, 1024, 16, 64
NCORES = 8
HPC = 4          # heads per core
GC = HPC * HD    # 256 columns per head group
NQ = 512         # q-tile width
KB = 128         # key block

_cache = {}


def _build(t_len):
    from concourse import bacc
    import concourse.tile as tile
    import concourse.mybir as mybir

    dt = mybir.dt
    f32, f32r, f16 = dt.float32, dt.float32r, dt.float16

    n_qt = t_len // NQ           # q tiles (and 512-wide n tiles)
    n_tt = t_len // KB           # token tiles of 128
    n_kc = D // KB               # contraction chunks over D (8)

    nc = bacc.Bacc("TRN2", debug=False, num_devices=NCORES)

    xt = nc.dram_tensor("XT", [D, t_len], f16, kind="ExternalInput")
    wq = nc.dram_tensor("WQ", [D, GC], f16, kind="ExternalInput")
    wk = nc.dram_tensor("WK", [D, GC], f16, kind="ExternalInput")
    wv = nc.dram_tensor("WV", [D, GC], f16, kind="ExternalInput")
    wo = nc.dram_tensor("WO", [GC, D], f16, kind="ExternalInput")
    ot = nc.dram_tensor("OT", [D, t_len], f16, kind="ExternalOutput")

    with tile.TileContext(nc) as tc:
        with (
            tc.tile_pool(name="w", bufs=1) as wpool,
            tc.tile_pool(name="proj", bufs=1) as proj,
            tc.tile_pool(name="pt", bufs=8) as ptpool,
            tc.tile_pool(name="small", bufs=6) as small,
            tc.tile_pool(name="ot", bufs=8) as otpool,
            tc.tile_pool(name="ps", bufs=2, space="PSUM") as pspool,
            tc.tile_pool(name="st", bufs=2, space="PSUM") as stpool,
            tc.tile_pool(name="ct", bufs=1, space="PSUM") as ctpool,
        ):
            # ---- load inputs (weights first so matmuls can start early) ----
            w_sb = {}

            def load_w(name, dram):
                t_ = wpool.tile([128, n_kc, GC], f16, tag=f"w{name}",
                                name=f"w{name}")
                nc.sync.dma_start(
                    out=t_, in_=dram.ap().rearrange("(c p) n -> p c n", p=128)
                )
                w_sb[name] = t_

            xt_sb = []

            def load_xt(c):
                t_ = wpool.tile([128, t_len], f16, tag=f"xt{c}",
                                name=f"xt{c}")
                nc.sync.dma_start(out=t_, in_=xt.ap()[c * 128:(c + 1) * 128, :])
                xt_sb.append(t_)

            load_xt(0)
            load_w("q", wq)
            load_xt(1)
            load_w("k", wk)
            for c in range(2, n_kc):
                load_xt(c)
            load_w("v", wv)
            wo_sb = wpool.tile([128, 2, D], f16, tag="wo")
            nc.sync.dma_start(
                out=wo_sb, in_=wo.ap().rearrange("(c p) n -> p c n", p=128)
            )

            qt_sb = [proj.tile([128, t_len], f16, tag=f"qt{m}", name=f"qt{m}")
                     for m in range(2)]
            kt_sb = [proj.tile([128, t_len], f16, tag=f"kt{m}", name=f"kt{m}")
                     for m in range(2)]
            v_sb = proj.tile([128, n_tt, HPC, HD + 1], f16, tag="v")
            nc.vector.memset(v_sb, 1.0)
            ct_sb = [proj.tile([128, t_len], f16, tag=f"ct{m}", name=f"ctn{m}")
                     for m in range(2)]
            diag_mask = proj.tile([128, 2, NQ], f16, tag="dmask")
            nc.vector.memset(diag_mask, 1.0)
            nc.gpsimd.affine_select(
                out=diag_mask,
                in_=diag_mask,
                compare_op=mybir.AluOpType.is_ge,
                fill=0.0,
                base=0,
                pattern=[[0, 2], [1, NQ]],
                channel_multiplier=-1,
            )

            def proj_slice(n, skip_qk=False):
                """Project QT/KT n-slice (both m tiles) + V token tiles."""
                for dst, wname in () if skip_qk else ((qt_sb, "q"), (kt_sb, "k")):
                    for m in range(2):
                        ps = pspool.tile([128, NQ], f32, tag="ps", name="ps")
                        for kc in range(n_kc):
                            nc.tensor.matmul(
                                ps,
                                w_sb[wname][:, kc, m * 128:(m + 1) * 128],
                                xt_sb[kc][:, n * NQ:(n + 1) * NQ],
                                start=(kc == 0),
                                stop=(kc == n_kc - 1),
                            )
                        nc.vector.tensor_copy(
                            dst[m][:, n * NQ:(n + 1) * NQ], ps
                        )
                for tt in range(4 * n, 4 * n + 4):
                    ps = pspool.tile([128, NQ], f32, tag="ps", name="ps")
                    for kc in range(n_kc):
                        nc.tensor.matmul(
                            ps[:, 0:GC],
                            xt_sb[kc][:, tt * 128:(tt + 1) * 128],
                            w_sb["v"][:, kc, :],
                            start=(kc == 0),
                            stop=(kc == n_kc - 1),
                        )
                    nc.vector.tensor_copy(
                        v_sb[:, tt, :, 0:HD],
                        ps[:, 0:GC].rearrange("p (h d) -> p h d", h=HPC),
                    )

            def attention(qt, mid=None):
                q0 = qt * NQ
                nkb = (q0 + NQ) // KB
                for hp in range(2):
                    if hp == 1 and mid is not None:
                        mid()
                    ct_ps = [
                        ctpool.tile([HD + 1, NQ], f32, tag=f"ct{i}",
                                    name=f"ctps{i}")
                        for i in range(2)
                    ]
                    for kb in range(nkb):
                        k0 = kb * KB
                        off = max(0, k0 - q0)   # fully-masked q prefix
                        w = NQ - off
                        diag = k0 + KB > q0
                        st = stpool.tile([128, 2 * NQ], f32, name="st")
                        st3 = st.rearrange("p (i q) -> p i q", i=2)
                        for i in range(2):
                            s_off = off
                            nc.tensor.matmul(
                                st3[:, i, s_off:],
                                kt_sb[hp][i * 64:(i + 1) * 64, k0:k0 + KB],
                                qt_sb[hp][i * 64:(i + 1) * 64,
                                          q0 + s_off:q0 + NQ],
                                start=True,
                                stop=True,
                                tile_position=(64 * i, 0),
                            )
                        pt = ptpool.tile([128, 2 * NQ], f16, name="pt")
                        pt3 = pt.rearrange("p (i q) -> p i q", i=2)
                        nc.scalar.activation(
                            out=pt3[:, :, off:], in_=st3[:, :, off:],
                            func=mybir.ActivationFunctionType.Exp,
                            scale=float(1.0 / np.sqrt(HD)),
                        )
                        if diag:  # zero where k > q (x < p in slice coords)
                            nc.vector.tensor_mul(
                                pt3[:, :, off:],
                                pt3[:, :, off:],
                                diag_mask[:, :, 0:w],
                            )
                        for i in range(2):
                            nc.tensor.matmul(
                                ct_ps[i][:, off:],
                                v_sb[:, kb, 2 * hp + i, :],
                                pt3[:, i, off:],
                                start=(kb == 0),
                                stop=(kb == nkb - 1),
                            )
                    # normalize: ct_norm = ct[0:64] * (1/l), l = ct row 64.
                    # Stage to SBUF quickly so the PSUM bank frees early.
                    for i in range(2):
                        stage = small.tile([HD, NQ], f32, tag="stage",
                                           name="stage")
                        nc.vector.tensor_copy(stage, ct_ps[i][0:HD, :])
                        l_sb = small.tile([1, NQ], f32, tag="l_sb",
                                          name="l_sb")
                        nc.scalar.copy(l_sb, ct_ps[i][HD:HD + 1, :])
                        lb = small.tile([HD, NQ], f32, tag="lb", name="lb")
                        nc.gpsimd.partition_broadcast(lb, l_sb)
                        rb = small.tile([HD, NQ], f32, tag="rb", name="rb")
                        nc.vector.reciprocal(out=rb, in_=lb)
                        nc.vector.tensor_mul(
                            ct_sb[hp][i * 64:(i + 1) * 64, q0:q0 + NQ],
                            stage,
                            rb,
                        )

            def out_proj(n):
                for m in range(n_kc):  # 8 dout tiles of 128
                    ps = pspool.tile([128, NQ], f32, tag="ps", name="ps")
                    for cc in range(2):
                        nc.tensor.matmul(
                            ps,
                            wo_sb[:, cc, m * 128:(m + 1) * 128],
                            ct_sb[cc][:, n * NQ:(n + 1) * NQ],
                            start=(cc == 0),
                            stop=(cc == 1),
                        )
                    o_sb = otpool.tile([128, NQ], f16, name="o_sb")
                    nc.vector.tensor_copy(o_sb, ps)
                    nc.sync.dma_start(
                        out=ot.ap()[m * 128:(m + 1) * 128,
                                    n * NQ:(n + 1) * NQ],
                        in_=o_sb,
                    )

            def proj_first_qk():
                """QT/KT n=0 via the idle st-pool banks: 4 accumulation
                groups in flight so the PE tracks XT chunk arrivals."""
                st_a = stpool.tile([128, 2 * NQ], f32, name="st")
                st_b = stpool.tile([128, 2 * NQ], f32, name="st")
                regions = [
                    (qt_sb, "q", 0, st_a[:, 0:NQ]),
                    (qt_sb, "q", 1, st_a[:, NQ:2 * NQ]),
                    (kt_sb, "k", 0, st_b[:, 0:NQ]),
                    (kt_sb, "k", 1, st_b[:, NQ:2 * NQ]),
                ]
                for kc in range(n_kc):
                    for dst, wname, m, reg in regions:
                        nc.tensor.matmul(
                            reg,
                            w_sb[wname][:, kc, m * 128:(m + 1) * 128],
                            xt_sb[kc][:, 0:NQ],
                            start=(kc == 0),
                            stop=(kc == n_kc - 1),
                        )
                for dst, wname, m, reg in regions:
                    nc.vector.tensor_copy(dst[m][:, 0:NQ], reg)

            # ---- pipelined schedule ----
            proj_first_qk()
            proj_slice(0, skip_qk=True)
            for qt in range(n_qt):
                if qt == n_qt - 1 and n_qt > 1:
                    attention(qt, mid=lambda: out_proj(0))
                    continue_emit = True
                else:
                    attention(qt)
                if qt + 1 < n_qt:
                    proj_slice(qt + 1)
                else:
                    for n in range(1 if n_qt > 1 else 0, n_qt):
                        out_proj(n)

    nc.compile()
    return nc


def get_nc(t_len=T):
    if t_len not in _cache:
        _cache[t_len] = _build(t_len)
    return _cache[t_len]


def make_in_maps(X, Wq, Wk, Wv, Wo):
    X = np.asarray(X, dtype=np.float32)
    Wq = np.asarray(Wq, dtype=np.float32)
    Wk = np.asarray(Wk, dtype=np.float32)
    Wv = np.asarray(Wv, dtype=np.float32)
    Wo = np.asarray(Wo, dtype=np.float32)
    in_maps = []
    for c in range(NCORES):
        b, g = divmod(c, 4)
        cols = slice(g * GC, (g + 1) * GC)
        in_maps.append({
            "XT": np.ascontiguousarray(X[b].T).astype(np.float16),
            "WQ": np.ascontiguousarray(Wq[:, cols]).astype(np.float16),
            "WK": np.ascontiguousarray(Wk[:, cols]).astype(np.float16),
            "WV": np.ascontiguousarray(Wv[:, cols]).astype(np.float16),
            "WO": np.ascontiguousarray(Wo[cols, :]).astype(np.float16),
        })
    return in_maps


def gather_out(results, bo):
    out = np.zeros((B, T, D), dtype=np.float32)
    for c in range(NCORES):
        out[c // 4] += results[c]["OT"].T.astype(np.float32)
    out += np.asarray(bo, dtype=np.float32)
    return out


def kernel(X, Wq, Wk, Wv, Wo, bo):
    from concourse import bass_utils

    nc = get_nc(T)
    in_maps = make_in_maps(X, Wq, Wk, Wv, Wo)
    res = bass_utils.run_bass_kernel_spmd(
        nc, in_maps, core_ids=list(range(NCORES))
    )
    return gather_out(res.results, bo)



# revision 2
# speedup vs baseline: 1.0009x; 1.0009x over previous
"""Multi-head causal attention (B=2, T=2048, D=1024, H=16) on 8 TRN2 cores.

Sharding: core c handles batch b=c//4 and head group g=c%4 (4 heads each).

Per-core device kernel (all fp16 matmuls, fp32 PSUM):
  QT/KT = (X Wq/Wk)^T in [head*64, T] layout (2 m-tiles of 128 = 2 heads)
  V_aug = X Wv in [T, h, 65] layout (col 64 = ones for softmax denominator)
  attention per (qt 512-q tile, head), kb chunks of 2x128 keys:
    ST[k, 2, q] = KT-vs-QT matmuls (PSUM, quadrant tile_position per head)
    PT = exp(ST/8) fp16; diag blocks masked via gpsimd affine_select
    CT2[q, qsub, 65] += PT^T-vs-V_aug matmuls (P stationary = 65 rows/blk,
      accumulated with start=False onto a gpsimd-memset PSUM bank)
    c2[q, qsub, c] = CT2[:, :, 0:64] * recip(CT2[:, :, 64]) (per-partition l)
  c2 -> PE transpose -> ct[c, q] -> OT[d, q] = WO-vs-ct matmuls -> DMA out
Projection slices n=1..3 and out-proj/transpose work run as micro-grained
filler between attention chunks (budgeted by the ACT-vs-PE deficit) so the
PE never stalls on the exp chain and its p-state stays ramped.
Host: out[b] = sum over the 4 head-group cores of OT^T, + bo.
"""

import numpy as np
from collections import deque

B, T, D, H, HD = 2, 2048, 1024, 16, 64
NCORES = 8
HPC = 4          # heads per core
GC = HPC * HD    # 256 columns per head group
NQ = 512         # q-tile width
KB = 128         # key block

_cache = {}

# schedule tunables (swept by tune2.py)
PARAMS = {
    "budgets": (180, 180, 180, 250),
    "final_pop": 300,
}


def _build(t_len):
    from concourse import bacc
    import concourse.tile as tile
    import concourse.mybir as mybir
    from concourse.masks import make_identity

    dt = mybir.dt
    f32, f16 = dt.float32, dt.float16

    n_qt = t_len // NQ           # q tiles of 512
    n_tt = t_len // KB           # token tiles of 128
    n_kc = D // KB               # contraction chunks over D (8)

    nc = bacc.Bacc("TRN2", debug=False, num_devices=NCORES)

    xt = nc.dram_tensor("XT", [D, t_len], f16, kind="ExternalInput")
    wq = nc.dram_tensor("WQ", [D, GC], f16, kind="ExternalInput")
    wk = nc.dram_tensor("WK", [D, GC], f16, kind="ExternalInput")
    wv = nc.dram_tensor("WV", [D, GC], f16, kind="ExternalInput")
    wo = nc.dram_tensor("WO", [GC, D], f16, kind="ExternalInput")
    ot = nc.dram_tensor("OT", [D, t_len], f16, kind="ExternalOutput")

    with tile.TileContext(nc) as tc:
        with (
            tc.tile_pool(name="w", bufs=1) as wpool,
            tc.tile_pool(name="pt", bufs=6) as ptpool,
            tc.tile_pool(name="c2", bufs=3) as c2pool,
            tc.tile_pool(name="csb", bufs=3) as csbpool,
            tc.tile_pool(name="small", bufs=8) as small,
            tc.tile_pool(name="st", bufs=2, space="PSUM") as stpool,
            tc.tile_pool(name="ct2", bufs=2, space="PSUM") as ctpool,
            tc.tile_pool(name="ps", bufs=2, space="PSUM") as pspool,
        ):
            # ---- persistent SBUF tiles ----
            xt_sb = wpool.tile([128, n_kc, t_len], f16, tag="xt")
            wq_sb = wpool.tile([128, n_kc, GC], f16, tag="wq")
            wk_sb = wpool.tile([128, n_kc, GC], f16, tag="wk")
            wv_sb = wpool.tile([128, n_kc, GC], f16, tag="wv")
            wo_sb = wpool.tile([128, 2, D], f16, tag="wo")
            qt_sb = wpool.tile([128, 2, t_len], f16, tag="qt")
            kt_sb = wpool.tile([128, 2, t_len], f16, tag="kt")
            v_sb = wpool.tile([128, n_tt, HPC, HD + 1], f16, tag="v")
            ident = wpool.tile([128, 128], f16, tag="ident")
            dmask = wpool.tile([128, NQ], f16, tag="dmask")

            nc.vector.memset(v_sb, 1.0)
            make_identity(nc, ident)
            nc.gpsimd.memset(dmask, 1.0)
            # dmask[p, x] = 1 if x >= p else 0 (keep q >= k on diag blocks)
            nc.gpsimd.affine_select(
                out=dmask,
                in_=dmask,
                compare_op=mybir.AluOpType.is_ge,
                fill=0.0,
                base=0,
                pattern=[[1, NQ]],
                channel_multiplier=-1,
            )

            # ---- input DMAs ----
            # XT loads in two halves: columns [0:1024] (all the init chase
            # needs) first, columns [1024:2048] mid-kernel.
            def dma_w(dst, src, c0, cn):
                nc.sync.dma_start(
                    out=dst[:, c0:c0 + cn, :],
                    in_=src.ap()[c0 * 128:(c0 + cn) * 128, :].rearrange(
                        "(c p) n -> p c n", p=128
                    ),
                )

            def dma_xt(c0, cn, lo, hi):
                nc.sync.dma_start(
                    out=xt_sb[:, c0:c0 + cn, lo:hi],
                    in_=xt.ap()[c0 * 128:(c0 + cn) * 128, lo:hi].rearrange(
                        "(c p) n -> p c n", p=128
                    ),
                )

            dma_xt(0, 1, 0, NQ)
            dma_w(wq_sb, wq, 0, 2)
            dma_w(wk_sb, wk, 0, 2)
            dma_xt(1, 1, 0, NQ)
            dma_xt(2, 2, 0, NQ)
            dma_w(wq_sb, wq, 2, 6)
            dma_w(wk_sb, wk, 2, 6)
            dma_xt(4, 2, 0, NQ)
            dma_xt(6, 2, 0, NQ)
            dma_w(wv_sb, wv, 0, 8)
            for c in range(0, n_kc, 2):
                dma_xt(c, 2, NQ, 2 * NQ)
            nc.sync.dma_start(
                out=wo_sb, in_=wo.ap().rearrange("(c p) n -> p c n", p=128)
            )
            for c in range(0, n_kc, 2):
                dma_xt(c, 2, 2 * NQ, t_len)

            # ---- init: QT/KT n=0, chasing the XT chunk DMAs ----
            def qk_init():
                stq = stpool.tile([128, 2, NQ], f32, tag="st", name="stq")
                stk = stpool.tile([128, 2, NQ], f32, tag="st", name="stk")
                for kc in range(n_kc):
                    for m in range(2):
                        nc.tensor.matmul(
                            stq[:, m, :],
                            wq_sb[:, kc, m * 128:(m + 1) * 128],
                            xt_sb[:, kc, 0:NQ],
                            start=(kc == 0), stop=(kc == n_kc - 1),
                        )
                        nc.tensor.matmul(
                            stk[:, m, :],
                            wk_sb[:, kc, m * 128:(m + 1) * 128],
                            xt_sb[:, kc, 0:NQ],
                            start=(kc == 0), stop=(kc == n_kc - 1),
                        )
                # split copies across DVE and ACT to shorten the handoff
                nc.vector.tensor_copy(qt_sb[:, 0, 0:NQ], stq[:, 0, :])
                nc.scalar.copy(qt_sb[:, 1, 0:NQ], stq[:, 1, :])
                nc.vector.tensor_copy(kt_sb[:, 0, 0:NQ], stk[:, 0, :])
                nc.scalar.copy(kt_sb[:, 1, 0:NQ], stk[:, 1, :])

            # ---- filler: generators yielding (approx_pe_ns, done_key|None) ----
            filler = deque()
            done = set()

            def pop_ns(budget):
                while budget > 0 and filler:
                    gen = filler[0]
                    try:
                        cost, key = next(gen)
                        budget -= cost
                        if key is not None:
                            done.add(key)
                    except StopIteration:
                        filler.popleft()
                return budget

            def require(key):
                """Drain filler until `key`'s unit has fully emitted."""
                while key not in done:
                    if not filler:
                        raise AssertionError(f"deadlock: {key} never emitted")
                    pop_ns(220)  # fine steps: stop right at the key

            def drain_filler():
                while filler:
                    pop_ns(10_000)

            MM_NS = 512 * 0.4167

            def proj_qk_unit(dst, wsb, m, n, key):
                def gen():
                    ps = pspool.tile([128, NQ], f32, tag="ps", name="ps")
                    for kc in range(n_kc):
                        nc.tensor.matmul(
                            ps,
                            wsb[:, kc, m * 128:(m + 1) * 128],
                            xt_sb[:, kc, n * NQ:(n + 1) * NQ],
                            start=(kc == 0), stop=(kc == n_kc - 1),
                        )
                        yield MM_NS, None
                    nc.vector.tensor_copy(dst[:, m, n * NQ:(n + 1) * NQ], ps)
                    yield 0, key
                return gen()

            def proj_v_unit(tt):
                def gen():
                    ps = pspool.tile([128, NQ], f32, tag="ps", name="ps")
                    for kc in range(n_kc):
                        nc.tensor.matmul(
                            ps[:, 0:GC],
                            xt_sb[:, kc, tt * 128:(tt + 1) * 128],
                            wv_sb[:, kc, :],
                            start=(kc == 0), stop=(kc == n_kc - 1),
                        )
                        yield MM_NS / 2, None
                    nc.vector.tensor_copy(
                        v_sb[:, tt, :, 0:HD],
                        ps[:, 0:GC].rearrange("p (h d) -> p h d", h=HPC),
                    )
                    yield 0, ("v", tt)
                return gen()

            c2_tiles = {}
            ct_tiles = {}

            def transp_emit(qt, i):
                if qt not in ct_tiles:
                    ct_tiles[qt] = csbpool.tile(
                        [128, 2, NQ], f16, tag="csb", name=f"ct{qt}"
                    )
                ctq = ct_tiles[qt]
                tp = ctpool.tile([128, 4, KB], f32, tag="ct2", name="tp")
                tpv = tp[:, :, :].rearrange("p a b -> p (a b)").bitcast(
                    f16).rearrange("p (a b) -> p a b", a=4)[:, :, 0:KB]
                c2 = c2_tiles[qt]
                for qs in range(4):
                    nc.tensor.transpose(
                        tpv[:, qs, :],
                        c2[:, qs, i * 128:(i + 1) * 128],
                        ident,
                    )
                nc.vector.tensor_copy(ctq[:, i, :], tpv)

            def outproj_unit(qt, m):
                def gen():
                    ctq = ct_tiles[qt]
                    ps = pspool.tile([128, NQ], f32, tag="ps", name="ps")
                    for cc in range(2):
                        nc.tensor.matmul(
                            ps,
                            wo_sb[:, cc, m * 128:(m + 1) * 128],
                            ctq[:, cc, :],
                            start=(cc == 0), stop=(cc == 1),
                        )
                        yield MM_NS, None
                    o_sb = small.tile([128, NQ], f16, tag="o_sb", name="o_sb")
                    nc.vector.tensor_copy(o_sb, ps)
                    nc.sync.dma_start(
                        out=ot.ap()[m * 128:(m + 1) * 128,
                                    qt * NQ:(qt + 1) * NQ],
                        in_=o_sb,
                    )
                    yield 0, None
                return gen()

            # ---- attention ----
            def attention(h, qt, pop_budget):
                q0 = qt * NQ
                nkb = (q0 + NQ) // KB
                sub, mt = h % 2, h // 2
                r0, r1 = sub * 64, sub * 64 + 64
                ct2 = ctpool.tile([128, 4, KB], f32, tag="ct2", name="ct2")
                pt_prev = None
                first_pv = [True]

                def pv(pt, chunk):
                    for j, kb in chunk:
                        qs_min = max(0, kb - 4 * qt)
                        for qs in range(qs_min, 4):
                            nc.tensor.matmul(
                                ct2[:, qs, 0:HD + 1],
                                pt[:, j, qs * 128:(qs + 1) * 128],
                                v_sb[:, kb, h, :],
                                start=first_pv[0], stop=False,
                                skip_group_check=True,
                            )
                            first_pv[0] = False

                chunks = [
                    [(j, c * 2 + j) for j in range(2)]
                    for c in range(nkb // 2)
                ]
                for c, chunk in enumerate(chunks):
                    require(("v", chunk[-1][1]))
                    st = stpool.tile([128, 2, NQ], f32, tag="st", name="st")
                    off0 = 0
                    for j, kb in chunk:
                        k0 = kb * KB
                        off = max(0, k0 - q0)
                        if j == 0:
                            off0 = off
                        nc.tensor.matmul(
                            st[:, j, off:],
                            kt_sb[r0:r1, mt, k0:k0 + KB],
                            qt_sb[r0:r1, mt, q0 + off:q0 + NQ],
                            start=True, stop=True,
                            tile_position=(r0, 0),
                        )
                    pt = ptpool.tile([128, 2, NQ], f16, tag="pt", name="pt")
                    nc.scalar.activation(
                        out=pt[:, :, off0:], in_=st[:, :, off0:],
                        func=mybir.ActivationFunctionType.Exp,
                        scale=float(1.0 / np.sqrt(HD)),
                    )
                    for j, kb in chunk:
                        off = max(0, kb * KB - q0)
                        if kb >= 4 * qt:  # diagonal block: zero k > q
                            nc.vector.tensor_mul(
                                pt[:, j, off:off + KB],
                                pt[:, j, off:off + KB],
                                dmask[:, 0:KB],
                            )
                    pop_ns(pop_budget)
                    if pt_prev is not None:
                        pv(*pt_prev)
                    pt_prev = (pt, chunk)
                pop_ns(PARAMS["final_pop"])
                pv(*pt_prev)
                # normalize: c2 = ct2[:, :, 0:64] * 1/l, l = ct2[:, :, 64]
                if h == 0:
                    c2_tiles[qt] = c2pool.tile(
                        [128, 4, GC], f16, tag="c2", name=f"c2_{qt}"
                    )
                c2 = c2_tiles[qt]
                rl = small.tile([128, 4], f32, tag="rl", name="rl")
                nc.vector.reciprocal(
                    out=rl,
                    in_=ct2[:, :, HD:HD + 1].rearrange("p a b -> p (a b)"),
                )
                for qs in range(4):
                    nc.vector.tensor_scalar(
                        out=c2[:, qs, h * HD:(h + 1) * HD],
                        in0=ct2[:, qs, 0:HD],
                        scalar1=rl[:, qs:qs + 1],
                        scalar2=None,
                        op0=mybir.AluOpType.mult,
                    )

            # ---- schedule ----
            qk_init()
            for m in range(2):
                done.add(("q", m, 0))
                done.add(("k", m, 0))
            for qt in range(n_qt):
                # supply filler for qt+1's attention; m=1 QK lags into
                # qt+1 itself (only needed once its h2 starts)
                n = qt + 1
                if qt == 0:
                    for tt in range(4):
                        filler.append(proj_v_unit(tt))
                if n < n_qt:
                    for dst, wsb, nm in ((qt_sb, wq_sb, "q"),
                                         (kt_sb, wk_sb, "k")):
                        filler.append(proj_qk_unit(dst, wsb, 0, n,
                                                   (nm, 0, n)))
                    for tt in range(4 * n, 4 * n + 4):
                        filler.append(proj_v_unit(tt))
                    for dst, wsb, nm in ((qt_sb, wq_sb, "q"),
                                         (kt_sb, wk_sb, "k")):
                        filler.append(proj_qk_unit(dst, wsb, 1, n,
                                                   (nm, 1, n)))
                if qt > 0:
                    for m in range(n_kc):
                        filler.append(outproj_unit(qt - 1, m))
                budget = PARAMS["budgets"][qt]
                for h in range(HPC):
                    if h == 2:
                        require(("q", 1, qt))
                        require(("k", 1, qt))
                        transp_emit(qt, 0)
                    attention(h, qt, budget)
                transp_emit(qt, 1)
                if qt + 1 < n_qt:
                    require(("q", 0, qt + 1))
                    require(("k", 0, qt + 1))
            for m in range(n_kc):
                filler.append(outproj_unit(n_qt - 1, m))
            drain_filler()

    nc.compile()
    return nc


def get_nc(t_len=T):
    if t_len not in _cache:
        _cache[t_len] = _build(t_len)
    return _cache[t_len]


def make_in_maps(X, Wq, Wk, Wv, Wo):
    X = np.asarray(X, dtype=np.float32)
    Wq = np.asarray(Wq, dtype=np.float32)
    Wk = np.asarray(Wk, dtype=np.float32)
    Wv = np.asarray(Wv, dtype=np.float32)
    Wo = np.asarray(Wo, dtype=np.float32)
    in_maps = []
    for c in range(NCORES):
        b, g = divmod(c, 4)
        cols = slice(g * GC, (g + 1) * GC)
        in_maps.append({
            "XT": np.ascontiguousarray(X[b].T).astype(np.float16),
            "WQ": np.ascontiguousarray(Wq[:, cols]).astype(np.float16),
            "WK": np.ascontiguousarray(Wk[:, cols]).astype(np.float16),
            "WV": np.ascontiguousarray(Wv[:, cols]).astype(np.float16),
            "WO": np.ascontiguousarray(Wo[cols, :]).astype(np.float16),
        })
    return in_maps


def gather_out(results, bo):
    out = np.zeros((B, T, D), dtype=np.float32)
    for c in range(NCORES):
        out[c // 4] += results[c]["OT"].T.astype(np.float32)
    out += np.asarray(bo, dtype=np.float32)
    return out


def kernel(X, Wq, Wk, Wv, Wo, bo):
    from concourse import bass_utils

    nc = get_nc(T)
    in_maps = make_in_maps(X, Wq, Wk, Wv, Wo)
    res = bass_utils.run_bass_kernel_spmd(
        nc, in_maps, core_ids=list(range(NCORES))
    )
    return gather_out(res.results, bo)


# revision 3
# speedup vs baseline: 1.0093x; 1.0085x over previous
"""Multi-head causal attention (B=2, T=2048, D=1024, H=16) on 8 TRN2 cores.

Sharding: core c handles batch b=c//4 and head group g=c%4 (4 heads each).

Per-core device kernel (all fp16 matmuls, fp32 PSUM):
  QT/KT = (X Wq/Wk)^T in [head*64, T] layout (2 m-tiles of 128 = 2 heads)
  V_aug = X Wv in [T, h, 65] layout (col 64 = ones for softmax denominator)
  attention per (qt 512-q tile, head), kb chunks of 2x128 keys:
    ST[k, 2, q] = KT-vs-QT matmuls (PSUM, quadrant tile_position per head)
    PT = exp(ST/8) fp16; diag blocks masked via gpsimd affine_select
    CT2[q, qsub, 65] += PT^T-vs-V_aug matmuls (P stationary = 65 rows/blk,
      accumulated with start=False onto a gpsimd-memset PSUM bank)
    c2[q, qsub, c] = CT2[:, :, 0:64] * recip(CT2[:, :, 64]) (per-partition l)
  c2 -> PE transpose -> ct[c, q] -> OT[d, q] = WO-vs-ct matmuls -> DMA out
Projection slices n=1..3 and out-proj/transpose work run as micro-grained
filler between attention chunks (budgeted by the ACT-vs-PE deficit) so the
PE never stalls on the exp chain and its p-state stays ramped.
Host: out[b] = sum over the 4 head-group cores of OT^T, + bo.
"""

import numpy as np
from collections import deque

B, T, D, H, HD = 2, 2048, 1024, 16, 64
NCORES = 8
HPC = 4          # heads per core
GC = HPC * HD    # 256 columns per head group
NQ = 512         # q-tile width
KB = 128         # key block

_cache = {}

# schedule tunables (swept by tune2.py)
PARAMS = {
    "budgets": (250, 250, 250, 250),
    "final_pop": 700,
}


def _build(t_len):
    from concourse import bacc
    import concourse.tile as tile
    import concourse.mybir as mybir
    from concourse.masks import make_identity

    dt = mybir.dt
    f32, f16 = dt.float32, dt.float16

    n_qt = t_len // NQ           # q tiles of 512
    n_tt = t_len // KB           # token tiles of 128
    n_kc = D // KB               # contraction chunks over D (8)

    nc = bacc.Bacc("TRN2", debug=False, num_devices=NCORES)

    xt = nc.dram_tensor("XT", [D, t_len], f16, kind="ExternalInput")
    wq = nc.dram_tensor("WQ", [D, GC], f16, kind="ExternalInput")
    wk = nc.dram_tensor("WK", [D, GC], f16, kind="ExternalInput")
    wv = nc.dram_tensor("WV", [D, GC], f16, kind="ExternalInput")
    wo = nc.dram_tensor("WO", [GC, D], f16, kind="ExternalInput")
    ot = nc.dram_tensor("OT", [D, t_len], f16, kind="ExternalOutput")
    ot2 = nc.dram_tensor("OT2", [D, NQ], f16, kind="ExternalOutput")

    with tile.TileContext(nc) as tc:
        with (
            tc.tile_pool(name="w", bufs=1) as wpool,
            tc.tile_pool(name="pt", bufs=6) as ptpool,
            tc.tile_pool(name="c2", bufs=3) as c2pool,
            tc.tile_pool(name="csb", bufs=3) as csbpool,
            tc.tile_pool(name="small", bufs=8) as small,
            tc.tile_pool(name="st", bufs=2, space="PSUM") as stpool,
            tc.tile_pool(name="ct2", bufs=2, space="PSUM") as ctpool,
            tc.tile_pool(name="ps", bufs=2, space="PSUM") as pspool,
        ):
            # ---- persistent SBUF tiles ----
            xt_sb = wpool.tile([128, n_kc, t_len], f16, tag="xt")
            wq_sb = wpool.tile([128, n_kc, GC], f16, tag="wq")
            wk_sb = wpool.tile([128, n_kc, GC], f16, tag="wk")
            wv_sb = wpool.tile([128, n_kc, GC], f16, tag="wv")
            wo_sb = wpool.tile([128, 2, D], f16, tag="wo")
            qt_sb = wpool.tile([128, 2, t_len], f16, tag="qt")
            kt_sb = wpool.tile([128, 2, t_len], f16, tag="kt")
            v_sb = wpool.tile([128, n_tt, HPC, HD + 1], f16, tag="v")
            ident = wpool.tile([128, 128], f16, tag="ident")
            dmask = wpool.tile([128, NQ], f16, tag="dmask")

            nc.vector.memset(v_sb, 1.0)
            make_identity(nc, ident)
            nc.gpsimd.memset(dmask, 1.0)
            # dmask[p, x] = 1 if x >= p else 0 (keep q >= k on diag blocks)
            nc.gpsimd.affine_select(
                out=dmask,
                in_=dmask,
                compare_op=mybir.AluOpType.is_ge,
                fill=0.0,
                base=0,
                pattern=[[1, NQ]],
                channel_multiplier=-1,
            )

            # ---- input DMAs ----
            # XT loads in two halves: columns [0:1024] (all the init chase
            # needs) first, columns [1024:2048] mid-kernel.
            def dma_w(dst, src, c0, cn):
                nc.sync.dma_start(
                    out=dst[:, c0:c0 + cn, :],
                    in_=src.ap()[c0 * 128:(c0 + cn) * 128, :].rearrange(
                        "(c p) n -> p c n", p=128
                    ),
                )

            def dma_xt(c0, cn, lo, hi):
                nc.sync.dma_start(
                    out=xt_sb[:, c0:c0 + cn, lo:hi],
                    in_=xt.ap()[c0 * 128:(c0 + cn) * 128, lo:hi].rearrange(
                        "(c p) n -> p c n", p=128
                    ),
                )

            dma_xt(0, 1, 0, NQ)
            dma_w(wq_sb, wq, 0, 2)
            dma_w(wk_sb, wk, 0, 2)
            dma_xt(1, 1, 0, NQ)
            dma_xt(2, 2, 0, NQ)
            dma_w(wq_sb, wq, 2, 6)
            dma_w(wk_sb, wk, 2, 6)
            dma_xt(4, 2, 0, NQ)
            dma_xt(6, 2, 0, NQ)
            dma_w(wv_sb, wv, 0, 8)
            for c in range(0, n_kc, 2):
                dma_xt(c, 2, NQ, 2 * NQ)
            nc.sync.dma_start(
                out=wo_sb, in_=wo.ap().rearrange("(c p) n -> p c n", p=128)
            )
            for c in range(0, n_kc, 2):
                dma_xt(c, 2, 2 * NQ, t_len)

            # ---- init: QT/KT n=0, chasing the XT chunk DMAs ----
            def qk_init():
                stq = stpool.tile([128, 2, NQ], f32, tag="st", name="stq")
                stk = stpool.tile([128, 2, NQ], f32, tag="st", name="stk")
                for kc in range(n_kc):
                    for m in range(2):
                        nc.tensor.matmul(
                            stq[:, m, :],
                            wq_sb[:, kc, m * 128:(m + 1) * 128],
                            xt_sb[:, kc, 0:NQ],
                            start=(kc == 0), stop=(kc == n_kc - 1),
                        )
                        nc.tensor.matmul(
                            stk[:, m, :],
                            wk_sb[:, kc, m * 128:(m + 1) * 128],
                            xt_sb[:, kc, 0:NQ],
                            start=(kc == 0), stop=(kc == n_kc - 1),
                        )
                # split copies across DVE and ACT to shorten the handoff
                nc.vector.tensor_copy(qt_sb[:, 0, 0:NQ], stq[:, 0, :])
                nc.scalar.copy(qt_sb[:, 1, 0:NQ], stq[:, 1, :])
                nc.vector.tensor_copy(kt_sb[:, 0, 0:NQ], stk[:, 0, :])
                nc.scalar.copy(kt_sb[:, 1, 0:NQ], stk[:, 1, :])

            # ---- filler: generators yielding (approx_pe_ns, done_key|None) ----
            filler = deque()
            done = set()

            starve = []

            def pop_ns(budget):
                while budget > 0 and filler:
                    gen = filler[0]
                    try:
                        cost, key = next(gen)
                        budget -= cost
                        if key is not None:
                            done.add(key)
                    except StopIteration:
                        filler.popleft()
                if budget > 0:
                    starve.append((tuple(where), budget))
                return budget

            def require(key):
                """Drain filler until `key`'s unit has fully emitted."""
                while key not in done:
                    if not filler:
                        raise AssertionError(f"deadlock: {key} never emitted")
                    pop_ns(220)  # fine steps: stop right at the key

            def drain_filler():
                while filler:
                    pop_ns(10_000)

            MM_NS = 512 * 0.4167

            def proj_qk_unit(dst, wsb, m, n, key):
                def gen():
                    ps = pspool.tile([128, NQ], f32, tag="ps", name="ps")
                    for kc in range(n_kc):
                        nc.tensor.matmul(
                            ps,
                            wsb[:, kc, m * 128:(m + 1) * 128],
                            xt_sb[:, kc, n * NQ:(n + 1) * NQ],
                            start=(kc == 0), stop=(kc == n_kc - 1),
                        )
                        yield MM_NS, None
                    nc.vector.tensor_copy(dst[:, m, n * NQ:(n + 1) * NQ], ps)
                    yield 0, key
                return gen()

            def proj_v_unit(tt):
                def gen():
                    ps = pspool.tile([128, NQ], f32, tag="ps", name="ps")
                    for kc in range(n_kc):
                        nc.tensor.matmul(
                            ps[:, 0:GC],
                            xt_sb[:, kc, tt * 128:(tt + 1) * 128],
                            wv_sb[:, kc, :],
                            start=(kc == 0), stop=(kc == n_kc - 1),
                        )
                        yield MM_NS / 2, None
                    nc.vector.tensor_copy(
                        v_sb[:, tt, :, 0:HD],
                        ps[:, 0:GC].rearrange("p (h d) -> p h d", h=HPC),
                    )
                    yield 0, ("v", tt)
                return gen()

            c2_tiles = {}
            ct_tiles = {}

            def transp_emit(qt, i):
                if qt not in ct_tiles:
                    ct_tiles[qt] = csbpool.tile(
                        [128, 2, NQ], f16, tag="csb", name=f"ct{qt}"
                    )
                ctq = ct_tiles[qt]
                tp = ctpool.tile([128, 4, KB], f32, tag="ct2", name="tp")
                tpv = tp[:, :, :].rearrange("p a b -> p (a b)").bitcast(
                    f16).rearrange("p (a b) -> p a b", a=4)[:, :, 0:KB]
                c2 = c2_tiles[qt]
                for qs in range(4):
                    nc.tensor.transpose(
                        tpv[:, qs, :],
                        c2[:, qs, i * 128:(i + 1) * 128],
                        ident,
                    )
                nc.vector.tensor_copy(ctq[:, i, :], tpv)

            def outproj_half0(qt, m):
                def gen():
                    ctq = ct_tiles[qt]
                    ps = pspool.tile([128, NQ], f32, tag="ps", name="ps")
                    nc.tensor.matmul(
                        ps, wo_sb[:, 0, m * 128:(m + 1) * 128],
                        ctq[:, 0, :], start=True, stop=True,
                    )
                    yield MM_NS, None
                    o_sb = small.tile([128, NQ], f16, tag="o_sb", name="o_sb")
                    nc.vector.tensor_copy(o_sb, ps)
                    nc.sync.dma_start(
                        out=ot2.ap()[m * 128:(m + 1) * 128, :],
                        in_=o_sb,
                    )
                    yield 0, None
                return gen()

            def outproj_half1(qt, m, dst=None):
                def gen():
                    ctq = ct_tiles[qt]
                    ps = (pspool.tile([128, NQ], f32, tag="ps", name="ps")
                          if dst is None else dst)
                    nc.tensor.matmul(
                        ps, wo_sb[:, 1, m * 128:(m + 1) * 128],
                        ctq[:, 1, :], start=True, stop=True,
                    )
                    yield MM_NS, None
                    o_sb = small.tile([128, NQ], f16, tag="o_sb", name="o_sb")
                    if m % 2 == 0:
                        nc.vector.tensor_copy(o_sb, ps)
                    else:
                        nc.scalar.copy(o_sb, ps)
                    nc.sync.dma_start(
                        out=ot.ap()[m * 128:(m + 1) * 128,
                                    qt * NQ:(qt + 1) * NQ],
                        in_=o_sb,
                    )
                    yield 0, None
                return gen()

            def outproj_unit(qt, m):
                def gen():
                    ctq = ct_tiles[qt]
                    ps = pspool.tile([128, NQ], f32, tag="ps", name="ps")
                    for cc in range(2):
                        nc.tensor.matmul(
                            ps,
                            wo_sb[:, cc, m * 128:(m + 1) * 128],
                            ctq[:, cc, :],
                            start=(cc == 0), stop=(cc == 1),
                        )
                        yield MM_NS, None
                    o_sb = small.tile([128, NQ], f16, tag="o_sb", name="o_sb")
                    nc.vector.tensor_copy(o_sb, ps)
                    nc.sync.dma_start(
                        out=ot.ap()[m * 128:(m + 1) * 128,
                                    qt * NQ:(qt + 1) * NQ],
                        in_=o_sb,
                    )
                    yield 0, None
                return gen()

            # ---- attention ----
            def attention(h, qt, pop_budget):
                q0 = qt * NQ
                nkb = (q0 + NQ) // KB
                sub, mt = h % 2, h // 2
                r0, r1 = sub * 64, sub * 64 + 64
                ct2 = ctpool.tile([128, 4, KB], f32, tag="ct2", name="ct2")
                pt_prev = None
                first_pv = [True]

                def pv(pt, chunk):
                    for j, kb in chunk:
                        qs_min = max(0, kb - 4 * qt)
                        for qs in range(qs_min, 4):
                            nc.tensor.matmul(
                                ct2[:, qs, 0:HD + 1],
                                pt[:, j, qs * 128:(qs + 1) * 128],
                                v_sb[:, kb, h, :],
                                start=first_pv[0], stop=False,
                                skip_group_check=True,
                            )
                            first_pv[0] = False

                chunks = [
                    [(j, c * 2 + j) for j in range(2)]
                    for c in range(nkb // 2)
                ]
                for c, chunk in enumerate(chunks):
                    where[2] = c
                    require(("v", chunk[-1][1]))
                    st = stpool.tile([128, 2, NQ], f32, tag="st", name="st")
                    off0 = 0
                    for j, kb in chunk:
                        k0 = kb * KB
                        off = max(0, k0 - q0)
                        if j == 0:
                            off0 = off
                        nc.tensor.matmul(
                            st[:, j, off:],
                            kt_sb[r0:r1, mt, k0:k0 + KB],
                            qt_sb[r0:r1, mt, q0 + off:q0 + NQ],
                            start=True, stop=True,
                            tile_position=(r0, 0),
                        )
                    pt = ptpool.tile([128, 2, NQ], f16, tag="pt", name="pt")
                    nc.scalar.activation(
                        out=pt[:, :, off0:], in_=st[:, :, off0:],
                        func=mybir.ActivationFunctionType.Exp,
                        scale=float(1.0 / np.sqrt(HD)),
                    )
                    for j, kb in chunk:
                        off = max(0, kb * KB - q0)
                        if kb >= 4 * qt:  # diagonal block: zero k > q
                            nc.vector.tensor_mul(
                                pt[:, j, off:off + KB],
                                pt[:, j, off:off + KB],
                                dmask[:, 0:KB],
                            )
                    pop_ns(pop_budget)
                    if pt_prev is not None:
                        pv(*pt_prev)
                    pt_prev = (pt, chunk)
                pop_ns(PARAMS["final_pop"])
                pv(*pt_prev)
                # normalize: c2 = ct2[:, :, 0:64] * 1/l, l = ct2[:, :, 64]
                if h == 0:
                    c2_tiles[qt] = c2pool.tile(
                        [128, 4, GC], f16, tag="c2", name=f"c2_{qt}"
                    )
                c2 = c2_tiles[qt]
                rl = small.tile([128, 4], f32, tag="rl", name="rl")
                nc.vector.reciprocal(
                    out=rl,
                    in_=ct2[:, :, HD:HD + 1].rearrange("p a b -> p (a b)"),
                )
                for qs in range(4):
                    nc.vector.tensor_scalar(
                        out=c2[:, qs, h * HD:(h + 1) * HD],
                        in0=ct2[:, qs, 0:HD],
                        scalar1=rl[:, qs:qs + 1],
                        scalar2=None,
                        op0=mybir.AluOpType.mult,
                    )

            # ---- schedule ----
            where = [0, 0, 0]
            qk_init()
            for m in range(2):
                done.add(("q", m, 0))
                done.add(("k", m, 0))
            for qt in range(n_qt):
                # supply filler for qt+1's attention; m=1 QK lags into
                # qt+1 itself (only needed once its h2 starts)
                n = qt + 1
                if qt == 0:
                    for tt in range(4):
                        filler.append(proj_v_unit(tt))
                if n < n_qt:
                    for dst, wsb, nm in ((qt_sb, wq_sb, "q"),
                                         (kt_sb, wk_sb, "k")):
                        filler.append(proj_qk_unit(dst, wsb, 0, n,
                                                   (nm, 0, n)))
                    for tt in range(4 * n, 4 * n + 4):
                        filler.append(proj_v_unit(tt))
                    for dst, wsb, nm in ((qt_sb, wq_sb, "q"),
                                         (kt_sb, wk_sb, "k")):
                        filler.append(proj_qk_unit(dst, wsb, 1, n,
                                                   (nm, 1, n)))
                if qt > 0:
                    for m in range(n_kc):
                        filler.append(outproj_unit(qt - 1, m))
                budget = PARAMS["budgets"][qt]
                for h in range(HPC):
                    where[0], where[1] = qt, h
                    if h == 2:
                        require(("q", 1, qt))
                        require(("k", 1, qt))
                        transp_emit(qt, 0)
                        if qt == n_qt - 1:
                            for m in range(n_kc):
                                filler.append(outproj_half0(qt, m))
                    attention(h, qt, budget)
                transp_emit(qt, 1)
                if qt + 1 < n_qt:
                    require(("q", 0, qt + 1))
                    require(("k", 0, qt + 1))
            # final 8 out-proj groups spread across all free psum banks
            sta = stpool.tile([128, 2, NQ], f32, tag="st", name="sta")
            stb = stpool.tile([128, 2, NQ], f32, tag="st", name="stb")
            cta = ctpool.tile([128, 4, KB], f32, tag="ct2", name="cta")
            ctb = ctpool.tile([128, 4, KB], f32, tag="ct2", name="ctb")
            dsts = [None, None, sta[:, 0, :], sta[:, 1, :],
                    stb[:, 0, :], stb[:, 1, :],
                    cta[:, :, :].rearrange("p a b -> p (a b)"),
                    ctb[:, :, :].rearrange("p a b -> p (a b)")]
            for m in range(n_kc):
                filler.append(outproj_half1(n_qt - 1, m, dsts[m]))
            drain_filler()
            if starve:
                import collections
                agg = collections.Counter()
                for w, b in starve:
                    agg[w[0]] += b
                print("STARVE by qt:", dict(agg), "first:",
                      starve[0][0] if starve else None)

    nc.compile()
    return nc


def get_nc(t_len=T):
    if t_len not in _cache:
        _cache[t_len] = _build(t_len)
    return _cache[t_len]


def make_in_maps(X, Wq, Wk, Wv, Wo):
    X = np.asarray(X, dtype=np.float32)
    Wq = np.asarray(Wq, dtype=np.float32)
    Wk = np.asarray(Wk, dtype=np.float32)
    Wv = np.asarray(Wv, dtype=np.float32)
    Wo = np.asarray(Wo, dtype=np.float32)
    in_maps = []
    for c in range(NCORES):
        b, g = divmod(c, 4)
        cols = slice(g * GC, (g + 1) * GC)
        in_maps.append({
            "XT": np.ascontiguousarray(X[b].T).astype(np.float16),
            "WQ": np.ascontiguousarray(Wq[:, cols]).astype(np.float16),
            "WK": np.ascontiguousarray(Wk[:, cols]).astype(np.float16),
            "WV": np.ascontiguousarray(Wv[:, cols]).astype(np.float16),
            "WO": np.ascontiguousarray(Wo[cols, :]).astype(np.float16),
        })
    return in_maps


def gather_out(results, bo):
    out = np.zeros((B, T, D), dtype=np.float32)
    for c in range(NCORES):
        out[c // 4] += results[c]["OT"].T.astype(np.float32)
        out[c // 4][T - NQ:] += results[c]["OT2"].T.astype(np.float32)
    out += np.asarray(bo, dtype=np.float32)
    return out


def kernel(X, Wq, Wk, Wv, Wo, bo):
    from concourse import bass_utils

    nc = get_nc(T)
    in_maps = make_in_maps(X, Wq, Wk, Wv, Wo)
    res = bass_utils.run_bass_kernel_spmd(
        nc, in_maps, core_ids=list(range(NCORES))
    )
    return gather_out(res.results, bo)


# revision 4
# speedup vs baseline: 1.0117x; 1.0024x over previous
"""Multi-head causal attention (B=2, T=2048, D=1024, H=16) on 8 TRN2 cores.

Sharding: core c handles batch b=c//4 and head group g=c%4 (4 heads each).

Per-core device kernel (all fp16 matmuls, fp32 PSUM):
  QT/KT = (X Wq/Wk)^T in [head*64, T] layout (2 m-tiles of 128 = 2 heads)
  V_aug = X Wv in [T, h, 65] layout (col 64 = ones for softmax denominator)
  attention per (qt 512-q tile, head), kb chunks of 2x128 keys:
    ST[k, 2, q] = KT-vs-QT matmuls (PSUM, quadrant tile_position per head)
    PT = exp(ST/8) fp16 (one ACT instr per 2-kb chunk); diag masked on DVE
    CT2[q, qsub, 65] += PT^T-vs-V_aug matmuls: P as the (free) stationary
      operand costs 65 rows/block vs 512 for the V-stationary form; the
      accumulation uses one start=True on the first matmul (whole-bank
      pending-zero) then start=False
    c2[q, qsub, c] = CT2[:, :, 0:64] * recip(CT2[:, :, 64]) (l on the
      q-partition axis, so no partition broadcast is needed); each head's
      final PV+normalize is deferred into the next head's stream
  c2 -> PE transpose (fp16 via bitcast PSUM view) -> ct[c, q]
  OT[d, q] = WO-vs-ct matmuls; the last q-tile splits the contraction in
  half across two output streams (OT/OT2, summed on host) so its first
  half runs mid-attention and the tail spreads over all 8 PSUM banks.
Projection slices n>=1 and out-proj work run as micro-grained generator
"filler" popped between attention chunks (~250ns/chunk, the ACT-vs-PE
deficit) so the PE never idles while ACT paces the exp chain; QT/KT n=0
is computed by 4 PSUM groups chasing the XT chunk DMAs at the front.
Host: out[b] = sum over the 4 head-group cores of OT^T (+OT2), + bo.
"""

import numpy as np
from collections import deque

B, T, D, H, HD = 2, 2048, 1024, 16, 64
NCORES = 8
HPC = 4          # heads per core
GC = HPC * HD    # 256 columns per head group
NQ = 512         # q-tile width
KB = 128         # key block

_cache = {}

# schedule tunables (swept by tune2.py)
PARAMS = {
    "budgets": (250, 250, 250, 250),
    "final_pop": 500,
}


def _build(t_len):
    from concourse import bacc
    import concourse.tile as tile
    import concourse.mybir as mybir
    from concourse.masks import make_identity

    dt = mybir.dt
    f32, f16 = dt.float32, dt.float16

    n_qt = t_len // NQ           # q tiles of 512
    n_tt = t_len // KB           # token tiles of 128
    n_kc = D // KB               # contraction chunks over D (8)

    nc = bacc.Bacc("TRN2", debug=False, num_devices=NCORES)

    xt = nc.dram_tensor("XT", [D, t_len], f16, kind="ExternalInput")
    wq = nc.dram_tensor("WQ", [D, GC], f16, kind="ExternalInput")
    wk = nc.dram_tensor("WK", [D, GC], f16, kind="ExternalInput")
    wv = nc.dram_tensor("WV", [D, GC], f16, kind="ExternalInput")
    wo = nc.dram_tensor("WO", [GC, D], f16, kind="ExternalInput")
    ot = nc.dram_tensor("OT", [D, t_len], f16, kind="ExternalOutput")
    ot2 = nc.dram_tensor("OT2", [D, NQ], f16, kind="ExternalOutput")

    with tile.TileContext(nc) as tc:
        with (
            tc.tile_pool(name="w", bufs=1) as wpool,
            tc.tile_pool(name="pt", bufs=6) as ptpool,
            tc.tile_pool(name="c2", bufs=3) as c2pool,
            tc.tile_pool(name="csb", bufs=3) as csbpool,
            tc.tile_pool(name="small", bufs=8) as small,
            tc.tile_pool(name="st", bufs=2, space="PSUM") as stpool,
            tc.tile_pool(name="ct2", bufs=2, space="PSUM") as ctpool,
            tc.tile_pool(name="ps", bufs=2, space="PSUM") as pspool,
        ):
            # ---- persistent SBUF tiles ----
            xt_sb = wpool.tile([128, n_kc, t_len], f16, tag="xt")
            wq_sb = wpool.tile([128, n_kc, GC], f16, tag="wq")
            wk_sb = wpool.tile([128, n_kc, GC], f16, tag="wk")
            wv_sb = wpool.tile([128, n_kc, GC], f16, tag="wv")
            wo_sb = wpool.tile([128, 2, D], f16, tag="wo")
            qt_sb = wpool.tile([128, 2, t_len], f16, tag="qt")
            kt_sb = wpool.tile([128, 2, t_len], f16, tag="kt")
            v_sb = wpool.tile([128, n_tt, HPC, HD + 1], f16, tag="v")
            ident = wpool.tile([128, 128], f16, tag="ident")
            dmask = wpool.tile([128, NQ], f16, tag="dmask")

            nc.vector.memset(v_sb, 1.0)
            make_identity(nc, ident)
            nc.gpsimd.memset(dmask, 1.0)
            # dmask[p, x] = 1 if x >= p else 0 (keep q >= k on diag blocks)
            nc.gpsimd.affine_select(
                out=dmask,
                in_=dmask,
                compare_op=mybir.AluOpType.is_ge,
                fill=0.0,
                base=0,
                pattern=[[1, NQ]],
                channel_multiplier=-1,
            )

            # ---- input DMAs ----
            # XT loads in two halves: columns [0:1024] (all the init chase
            # needs) first, columns [1024:2048] mid-kernel.
            def dma_w(dst, src, c0, cn):
                nc.sync.dma_start(
                    out=dst[:, c0:c0 + cn, :],
                    in_=src.ap()[c0 * 128:(c0 + cn) * 128, :].rearrange(
                        "(c p) n -> p c n", p=128
                    ),
                )

            def dma_xt(c0, cn, lo, hi):
                nc.sync.dma_start(
                    out=xt_sb[:, c0:c0 + cn, lo:hi],
                    in_=xt.ap()[c0 * 128:(c0 + cn) * 128, lo:hi].rearrange(
                        "(c p) n -> p c n", p=128
                    ),
                )

            dma_xt(0, 1, 0, NQ // 2)
            dma_w(wq_sb, wq, 0, 2)
            dma_w(wk_sb, wk, 0, 2)
            dma_xt(0, 1, NQ // 2, NQ)
            dma_xt(1, 1, 0, NQ)
            dma_xt(2, 2, 0, NQ)
            dma_w(wq_sb, wq, 2, 6)
            dma_w(wk_sb, wk, 2, 6)
            dma_xt(4, 2, 0, NQ)
            dma_xt(6, 2, 0, NQ)
            dma_w(wv_sb, wv, 0, 8)
            for c in range(0, n_kc, 2):
                dma_xt(c, 2, NQ, 2 * NQ)
            nc.sync.dma_start(
                out=wo_sb, in_=wo.ap().rearrange("(c p) n -> p c n", p=128)
            )
            for c in range(0, n_kc, 2):
                dma_xt(c, 2, 2 * NQ, t_len)

            # ---- init: QT/KT n=0, chasing the XT chunk DMAs ----
            def qk_init():
                stq = stpool.tile([128, 2, NQ], f32, tag="st", name="stq")
                stk = stpool.tile([128, 2, NQ], f32, tag="st", name="stk")
                for kc in range(n_kc):
                    cols = ((0, NQ // 2), (NQ // 2, NQ)) if kc == 0 \
                        else ((0, NQ),)
                    for ci, (lo, hi) in enumerate(cols):
                        for m in range(2):
                            nc.tensor.matmul(
                                stq[:, m, lo:hi],
                                wq_sb[:, kc, m * 128:(m + 1) * 128],
                                xt_sb[:, kc, lo:hi],
                                start=(kc == 0 and ci == 0),
                                stop=(kc == n_kc - 1),
                                skip_group_check=True,
                            )
                            nc.tensor.matmul(
                                stk[:, m, lo:hi],
                                wk_sb[:, kc, m * 128:(m + 1) * 128],
                                xt_sb[:, kc, lo:hi],
                                start=(kc == 0 and ci == 0),
                                stop=(kc == n_kc - 1),
                                skip_group_check=True,
                            )
                # split copies across DVE and ACT to shorten the handoff
                nc.vector.tensor_copy(qt_sb[:, 0, 0:NQ], stq[:, 0, :])
                nc.scalar.copy(qt_sb[:, 1, 0:NQ], stq[:, 1, :])
                nc.vector.tensor_copy(kt_sb[:, 0, 0:NQ], stk[:, 0, :])
                nc.scalar.copy(kt_sb[:, 1, 0:NQ], stk[:, 1, :])

            # ---- filler: generators yielding (approx_pe_ns, done_key|None) ----
            filler = deque()
            done = set()

            def pop_ns(budget):
                while budget > 0 and filler:
                    gen = filler[0]
                    try:
                        cost, key = next(gen)
                        budget -= cost
                        if key is not None:
                            done.add(key)
                    except StopIteration:
                        filler.popleft()
                return budget

            def require(key):
                """Drain filler until `key`'s unit has fully emitted."""
                while key not in done:
                    if not filler:
                        raise AssertionError(f"deadlock: {key} never emitted")
                    pop_ns(220)  # fine steps: stop right at the key

            def drain_filler():
                while filler:
                    pop_ns(10_000)

            MM_NS = 512 * 0.4167

            def proj_qk_unit(dst, wsb, m, n, key):
                def gen():
                    ps = pspool.tile([128, NQ], f32, tag="ps", name="ps")
                    for kc in range(n_kc):
                        nc.tensor.matmul(
                            ps,
                            wsb[:, kc, m * 128:(m + 1) * 128],
                            xt_sb[:, kc, n * NQ:(n + 1) * NQ],
                            start=(kc == 0), stop=(kc == n_kc - 1),
                        )
                        yield MM_NS, None
                    nc.vector.tensor_copy(dst[:, m, n * NQ:(n + 1) * NQ], ps)
                    yield 0, key
                return gen()

            def proj_v_unit(tt):
                def gen():
                    ps = pspool.tile([128, NQ], f32, tag="ps", name="ps")
                    for kc in range(n_kc):
                        nc.tensor.matmul(
                            ps[:, 0:GC],
                            xt_sb[:, kc, tt * 128:(tt + 1) * 128],
                            wv_sb[:, kc, :],
                            start=(kc == 0), stop=(kc == n_kc - 1),
                        )
                        yield MM_NS / 2, None
                    nc.vector.tensor_copy(
                        v_sb[:, tt, :, 0:HD],
                        ps[:, 0:GC].rearrange("p (h d) -> p h d", h=HPC),
                    )
                    yield 0, ("v", tt)
                return gen()

            c2_tiles = {}
            ct_tiles = {}

            def transp_emit(qt, i):
                if qt not in ct_tiles:
                    ct_tiles[qt] = csbpool.tile(
                        [128, 2, NQ], f16, tag="csb", name=f"ct{qt}"
                    )
                ctq = ct_tiles[qt]
                tp = ctpool.tile([128, 4, KB], f32, tag="ct2", name="tp")
                tpv = tp[:, :, :].rearrange("p a b -> p (a b)").bitcast(
                    f16).rearrange("p (a b) -> p a b", a=4)[:, :, 0:KB]
                c2 = c2_tiles[qt]
                for qs in range(4):
                    nc.tensor.transpose(
                        tpv[:, qs, :],
                        c2[:, qs, i * 128:(i + 1) * 128],
                        ident,
                    )
                nc.vector.tensor_copy(ctq[:, i, :], tpv)

            def outproj_half0(qt, m):
                def gen():
                    ctq = ct_tiles[qt]
                    ps = pspool.tile([128, NQ], f32, tag="ps", name="ps")
                    nc.tensor.matmul(
                        ps, wo_sb[:, 0, m * 128:(m + 1) * 128],
                        ctq[:, 0, :], start=True, stop=True,
                    )
                    yield MM_NS, None
                    o_sb = small.tile([128, NQ], f16, tag="o_sb", name="o_sb")
                    nc.vector.tensor_copy(o_sb, ps)
                    nc.sync.dma_start(
                        out=ot2.ap()[m * 128:(m + 1) * 128, :],
                        in_=o_sb,
                    )
                    yield 0, None
                return gen()

            def outproj_half1(qt, m, dst=None):
                def gen():
                    ctq = ct_tiles[qt]
                    ps = (pspool.tile([128, NQ], f32, tag="ps", name="ps")
                          if dst is None else dst)
                    nc.tensor.matmul(
                        ps, wo_sb[:, 1, m * 128:(m + 1) * 128],
                        ctq[:, 1, :], start=True, stop=True,
                    )
                    yield MM_NS, None
                    o_sb = small.tile([128, NQ], f16, tag="o_sb", name="o_sb")
                    if m % 2 == 0:
                        nc.vector.tensor_copy(o_sb, ps)
                    else:
                        nc.scalar.copy(o_sb, ps)
                    nc.sync.dma_start(
                        out=ot.ap()[m * 128:(m + 1) * 128,
                                    qt * NQ:(qt + 1) * NQ],
                        in_=o_sb,
                    )
                    yield 0, None
                return gen()

            def outproj_unit(qt, m):
                def gen():
                    ctq = ct_tiles[qt]
                    ps = pspool.tile([128, NQ], f32, tag="ps", name="ps")
                    for cc in range(2):
                        nc.tensor.matmul(
                            ps,
                            wo_sb[:, cc, m * 128:(m + 1) * 128],
                            ctq[:, cc, :],
                            start=(cc == 0), stop=(cc == 1),
                        )
                        yield MM_NS, None
                    o_sb = small.tile([128, NQ], f16, tag="o_sb", name="o_sb")
                    nc.vector.tensor_copy(o_sb, ps)
                    nc.sync.dma_start(
                        out=ot.ap()[m * 128:(m + 1) * 128,
                                    qt * NQ:(qt + 1) * NQ],
                        in_=o_sb,
                    )
                    yield 0, None
                return gen()

            # ---- attention ----
            finish_prev = [None]

            def attention(h, qt, pop_budget):
                q0 = qt * NQ
                nkb = (q0 + NQ) // KB
                sub, mt = h % 2, h // 2
                r0, r1 = sub * 64, sub * 64 + 64
                ct2 = ctpool.tile([128, 4, KB], f32, tag="ct2", name="ct2")
                pt_prev = None
                first_pv = [True]

                def pv(pt, chunk):
                    for j, kb in chunk:
                        qs_min = max(0, kb - 4 * qt)
                        for qs in range(qs_min, 4):
                            nc.tensor.matmul(
                                ct2[:, qs, 0:HD + 1],
                                pt[:, j, qs * 128:(qs + 1) * 128],
                                v_sb[:, kb, h, :],
                                start=first_pv[0], stop=False,
                                skip_group_check=True,
                            )
                            first_pv[0] = False

                chunks = [
                    [(j, c * 2 + j) for j in range(2)]
                    for c in range(nkb // 2)
                ]
                for c, chunk in enumerate(chunks):
                    where[2] = c
                    require(("v", chunk[-1][1]))
                    st = stpool.tile([128, 2, NQ], f32, tag="st", name="st")
                    off0 = 0
                    for j, kb in chunk:
                        k0 = kb * KB
                        off = max(0, k0 - q0)
                        if j == 0:
                            off0 = off
                        nc.tensor.matmul(
                            st[:, j, off:],
                            kt_sb[r0:r1, mt, k0:k0 + KB],
                            qt_sb[r0:r1, mt, q0 + off:q0 + NQ],
                            start=True, stop=True,
                            tile_position=(r0, 0),
                        )
                    pt = ptpool.tile([128, 2, NQ], f16, tag="pt", name="pt")
                    nc.scalar.activation(
                        out=pt[:, :, off0:], in_=st[:, :, off0:],
                        func=mybir.ActivationFunctionType.Exp,
                        scale=float(1.0 / np.sqrt(HD)),
                    )
                    for j, kb in chunk:
                        off = max(0, kb * KB - q0)
                        if kb >= 4 * qt:  # diagonal block: zero k > q
                            nc.vector.tensor_mul(
                                pt[:, j, off:off + KB],
                                pt[:, j, off:off + KB],
                                dmask[:, 0:KB],
                            )
                    if c == 0 and finish_prev[0] is not None:
                        finish_prev[0]()
                        finish_prev[0] = None
                    pop_ns(pop_budget)
                    if pt_prev is not None:
                        pv(*pt_prev)
                    pt_prev = (pt, chunk)

                def finish():
                    pop_ns(PARAMS["final_pop"])
                    pv(*pt_prev)
                    # c2 = ct2[:, :, 0:64] * 1/l, l = ct2[:, :, 64]
                    c2 = c2_tiles[qt]
                    rl = small.tile([128, 4], f32, tag="rl", name="rl")
                    nc.vector.reciprocal(
                        out=rl,
                        in_=ct2[:, :, HD:HD + 1].rearrange(
                            "p a b -> p (a b)"),
                    )
                    for qs in range(4):
                        nc.vector.tensor_scalar(
                            out=c2[:, qs, h * HD:(h + 1) * HD],
                            in0=ct2[:, qs, 0:HD],
                            scalar1=rl[:, qs:qs + 1],
                            scalar2=None,
                            op0=mybir.AluOpType.mult,
                        )
                if h == 0:
                    c2_tiles[qt] = c2pool.tile(
                        [128, 4, GC], f16, tag="c2", name=f"c2_{qt}"
                    )
                finish_prev[0] = finish

            # ---- schedule ----
            where = [0, 0, 0]
            qk_init()
            for m in range(2):
                done.add(("q", m, 0))
                done.add(("k", m, 0))
            for qt in range(n_qt):
                # supply filler for qt+1's attention; m=1 QK lags into
                # qt+1 itself (only needed once its h2 starts)
                n = qt + 1
                if qt == 0:
                    for tt in range(4):
                        filler.append(proj_v_unit(tt))
                if n < n_qt:
                    for dst, wsb, nm in ((qt_sb, wq_sb, "q"),
                                         (kt_sb, wk_sb, "k")):
                        filler.append(proj_qk_unit(dst, wsb, 0, n,
                                                   (nm, 0, n)))
                    for tt in range(4 * n, 4 * n + 4):
                        filler.append(proj_v_unit(tt))
                    for dst, wsb, nm in ((qt_sb, wq_sb, "q"),
                                         (kt_sb, wk_sb, "k")):
                        filler.append(proj_qk_unit(dst, wsb, 1, n,
                                                   (nm, 1, n)))
                if qt > 0:
                    for m in range(n_kc):
                        filler.append(outproj_unit(qt - 1, m))
                budget = PARAMS["budgets"][qt]
                for h in range(HPC):
                    where[0], where[1] = qt, h
                    if h == 2:
                        require(("q", 1, qt))
                        require(("k", 1, qt))
                    if h == 3:
                        transp_emit(qt, 0)
                        if qt == n_qt - 1:
                            for m in range(n_kc):
                                filler.append(outproj_half0(qt, m))
                    attention(h, qt, budget)
                if finish_prev[0] is not None:
                    finish_prev[0]()
                    finish_prev[0] = None
                transp_emit(qt, 1)
                if qt + 1 < n_qt:
                    require(("q", 0, qt + 1))
                    require(("k", 0, qt + 1))
            # final 8 out-proj groups spread across all free psum banks
            sta = stpool.tile([128, 2, NQ], f32, tag="st", name="sta")
            stb = stpool.tile([128, 2, NQ], f32, tag="st", name="stb")
            cta = ctpool.tile([128, 4, KB], f32, tag="ct2", name="cta")
            ctb = ctpool.tile([128, 4, KB], f32, tag="ct2", name="ctb")
            dsts = [None, None, sta[:, 0, :], sta[:, 1, :],
                    stb[:, 0, :], stb[:, 1, :],
                    cta[:, :, :].rearrange("p a b -> p (a b)"),
                    ctb[:, :, :].rearrange("p a b -> p (a b)")]
            for m in range(n_kc):
                filler.append(outproj_half1(n_qt - 1, m, dsts[m]))
            drain_filler()

    nc.compile()
    return nc


def get_nc(t_len=T):
    if t_len not in _cache:
        _cache[t_len] = _build(t_len)
    return _cache[t_len]


def make_in_maps(X, Wq, Wk, Wv, Wo):
    X = np.asarray(X, dtype=np.float32)
    Wq = np.asarray(Wq, dtype=np.float32)
    Wk = np.asarray(Wk, dtype=np.float32)
    Wv = np.asarray(Wv, dtype=np.float32)
    Wo = np.asarray(Wo, dtype=np.float32)
    in_maps = []
    for c in range(NCORES):
        b, g = divmod(c, 4)
        cols = slice(g * GC, (g + 1) * GC)
        in_maps.append({
            "XT": np.ascontiguousarray(X[b].T).astype(np.float16),
            "WQ": np.ascontiguousarray(Wq[:, cols]).astype(np.float16),
            "WK": np.ascontiguousarray(Wk[:, cols]).astype(np.float16),
            "WV": np.ascontiguousarray(Wv[:, cols]).astype(np.float16),
            "WO": np.ascontiguousarray(Wo[cols, :]).astype(np.float16),
        })
    return in_maps


def gather_out(results, bo):
    out = np.zeros((B, T, D), dtype=np.float32)
    for c in range(NCORES):
        out[c // 4] += results[c]["OT"].T.astype(np.float32)
        out[c // 4][T - NQ:] += results[c]["OT2"].T.astype(np.float32)
    out += np.asarray(bo, dtype=np.float32)
    return out


def kernel(X, Wq, Wk, Wv, Wo, bo):
    from concourse import bass_utils

    nc = get_nc(T)
    in_maps = make_in_maps(X, Wq, Wk, Wv, Wo)
    res = bass_utils.run_bass_kernel_spmd(
        nc, in_maps, core_ids=list(range(NCORES))
    )
    return gather_out(res.results, bo)


# revision 5
# speedup vs baseline: 1.0164x; 1.0046x over previous
"""Multi-head causal attention (B=2, T=2048, D=1024, H=16) on 8 TRN2 cores.

Sharding: core c handles batch b=c//4 and head group g=c%4 (4 heads each).

Per-core device kernel (all fp16 matmuls, fp32 PSUM):
  QT/KT = (X Wq/Wk)^T in [head*64, T] layout (2 m-tiles of 128 = 2 heads)
  V_aug = X Wv in [T, h, 65] layout (col 64 = ones for softmax denominator)
  attention per (qt 512-q tile, head), kb chunks of 2x128 keys:
    ST[k, 2, q] = KT-vs-QT matmuls (PSUM, quadrant tile_position per head)
    PT = exp(ST/8) fp16 (one ACT instr per 2-kb chunk); diag masked on DVE
    CT2[q, qsub, 65] += PT^T-vs-V_aug matmuls: P as the (free) stationary
      operand costs 65 rows/block vs 512 for the V-stationary form; the
      accumulation uses one start=True on the first matmul (whole-bank
      pending-zero) then start=False
    c2[q, qsub, c] = CT2[:, :, 0:64] * recip(CT2[:, :, 64]) (l on the
      q-partition axis, so no partition broadcast is needed); each head's
      final PV+normalize is deferred into the next head's stream
  c2 -> PE transpose (fp16 via bitcast PSUM view) -> ct[c, q]
  OT[d, q] = WO-vs-ct matmuls; the last q-tile splits the contraction in
  half across two output streams (OT/OT2, summed on host) so its first
  half runs mid-attention and the tail spreads over all 8 PSUM banks.
Projection slices n>=1 and out-proj work run as micro-grained generator
"filler" popped between attention chunks (~250ns/chunk, the ACT-vs-PE
deficit) so the PE never idles while ACT paces the exp chain; QT/KT n=0
is computed by 4 PSUM groups chasing the XT chunk DMAs at the front.
Host: out[b] = sum over the 4 head-group cores of OT^T (+OT2), + bo.
"""

import numpy as np
from collections import deque

B, T, D, H, HD = 2, 2048, 1024, 16, 64
NCORES = 8
HPC = 4          # heads per core
GC = HPC * HD    # 256 columns per head group
NQ = 512         # q-tile width
KB = 128         # key block

_cache = {}

# schedule tunables (swept by tune2.py)
PARAMS = {
    "budgets": (250, 250, 250, 250),
    "final_pop": 500,
}


def _build(t_len):
    from concourse import bacc
    import concourse.tile as tile
    import concourse.mybir as mybir
    from concourse.masks import make_identity

    dt = mybir.dt
    f32, f16 = dt.float32, dt.float16

    n_qt = t_len // NQ           # q tiles of 512
    n_tt = t_len // KB           # token tiles of 128
    n_kc = D // KB               # contraction chunks over D (8)

    nc = bacc.Bacc("TRN2", debug=False, num_devices=NCORES)

    xt = nc.dram_tensor("XT", [D, t_len], f16, kind="ExternalInput")
    wq = nc.dram_tensor("WQ", [D, GC], f16, kind="ExternalInput")
    wk = nc.dram_tensor("WK", [D, GC], f16, kind="ExternalInput")
    wv = nc.dram_tensor("WV", [D, GC], f16, kind="ExternalInput")
    wo = nc.dram_tensor("WO", [GC, D], f16, kind="ExternalInput")
    ot = nc.dram_tensor("OT", [D, t_len], f16, kind="ExternalOutput")
    ot2 = nc.dram_tensor("OT2", [D, NQ], f16, kind="ExternalOutput")

    with tile.TileContext(nc) as tc:
        with (
            tc.tile_pool(name="w", bufs=1) as wpool,
            tc.tile_pool(name="pt", bufs=6) as ptpool,
            tc.tile_pool(name="c2", bufs=3) as c2pool,
            tc.tile_pool(name="csb", bufs=3) as csbpool,
            tc.tile_pool(name="small", bufs=8) as small,
            tc.tile_pool(name="st", bufs=2, space="PSUM") as stpool,
            tc.tile_pool(name="ct2", bufs=2, space="PSUM") as ctpool,
            tc.tile_pool(name="ps", bufs=2, space="PSUM") as pspool,
        ):
            # ---- persistent SBUF tiles ----
            xt_sb = wpool.tile([128, n_kc, t_len], f16, tag="xt")
            wq_sb = wpool.tile([128, n_kc, GC], f16, tag="wq")
            wk_sb = wpool.tile([128, n_kc, GC], f16, tag="wk")
            wv_sb = wpool.tile([128, n_kc, GC], f16, tag="wv")
            wo_sb = wpool.tile([128, 2, D], f16, tag="wo")
            qt_sb = wpool.tile([128, 2, t_len], f16, tag="qt")
            kt_sb = wpool.tile([128, 2, t_len], f16, tag="kt")
            v_sb = wpool.tile([128, n_tt, HPC, HD + 1], f16, tag="v")
            ident = wpool.tile([128, 128], f16, tag="ident")
            dmask = wpool.tile([128, NQ], f16, tag="dmask")

            nc.vector.memset(v_sb, 1.0)
            make_identity(nc, ident)
            nc.gpsimd.memset(dmask, 1.0)
            # dmask[p, x] = 1 if x >= p else 0 (keep q >= k on diag blocks)
            nc.gpsimd.affine_select(
                out=dmask,
                in_=dmask,
                compare_op=mybir.AluOpType.is_ge,
                fill=0.0,
                base=0,
                pattern=[[1, NQ]],
                channel_multiplier=-1,
            )

            # ---- input DMAs ----
            # XT loads in two halves: columns [0:1024] (all the init chase
            # needs) first, columns [1024:2048] mid-kernel.
            def dma_w(dst, src, c0, cn):
                nc.sync.dma_start(
                    out=dst[:, c0:c0 + cn, :],
                    in_=src.ap()[c0 * 128:(c0 + cn) * 128, :].rearrange(
                        "(c p) n -> p c n", p=128
                    ),
                )

            def dma_xt(c0, cn, lo, hi):
                nc.sync.dma_start(
                    out=xt_sb[:, c0:c0 + cn, lo:hi],
                    in_=xt.ap()[c0 * 128:(c0 + cn) * 128, lo:hi].rearrange(
                        "(c p) n -> p c n", p=128
                    ),
                )

            dma_xt(0, 1, 0, NQ // 2)
            dma_w(wq_sb, wq, 0, 2)
            dma_w(wk_sb, wk, 0, 2)
            dma_xt(0, 1, NQ // 2, NQ)
            dma_xt(1, 1, 0, NQ)
            dma_xt(2, 2, 0, NQ)
            dma_w(wq_sb, wq, 2, 6)
            dma_w(wk_sb, wk, 2, 6)
            dma_xt(4, 2, 0, NQ)
            dma_xt(6, 2, 0, NQ)
            dma_w(wv_sb, wv, 0, 8)
            for c in range(0, n_kc, 2):
                dma_xt(c, 2, NQ, 2 * NQ)
            nc.sync.dma_start(
                out=wo_sb, in_=wo.ap().rearrange("(c p) n -> p c n", p=128)
            )
            for c in range(0, n_kc, 2):
                dma_xt(c, 2, 2 * NQ, t_len)

            # ---- init: QT/KT n=0, chasing the XT chunk DMAs ----
            def qk_init():
                stq = stpool.tile([128, 2, NQ], f32, tag="st", name="stq")
                stk = stpool.tile([128, 2, NQ], f32, tag="st", name="stk")
                for kc in range(n_kc):
                    cols = ((0, NQ // 2), (NQ // 2, NQ)) if kc == 0 \
                        else ((0, NQ),)
                    for ci, (lo, hi) in enumerate(cols):
                        for m in range(2):
                            nc.tensor.matmul(
                                stq[:, m, lo:hi],
                                wq_sb[:, kc, m * 128:(m + 1) * 128],
                                xt_sb[:, kc, lo:hi],
                                start=(kc == 0 and ci == 0),
                                stop=(kc == n_kc - 1),
                                skip_group_check=True,
                            )
                            nc.tensor.matmul(
                                stk[:, m, lo:hi],
                                wk_sb[:, kc, m * 128:(m + 1) * 128],
                                xt_sb[:, kc, lo:hi],
                                start=(kc == 0 and ci == 0),
                                stop=(kc == n_kc - 1),
                                skip_group_check=True,
                            )
                # split copies across DVE and ACT to shorten the handoff
                nc.vector.tensor_copy(qt_sb[:, 0, 0:NQ], stq[:, 0, :])
                nc.scalar.copy(qt_sb[:, 1, 0:NQ], stq[:, 1, :])
                nc.vector.tensor_copy(kt_sb[:, 0, 0:NQ], stk[:, 0, :])
                nc.scalar.copy(kt_sb[:, 1, 0:NQ], stk[:, 1, :])

            # ---- filler: generators yielding (approx_pe_ns, done_key|None) ----
            filler = deque()
            done = set()

            def pop_ns(budget):
                while budget > 0 and filler:
                    gen = filler[0]
                    try:
                        cost, key = next(gen)
                        budget -= cost
                        if key is not None:
                            done.add(key)
                    except StopIteration:
                        filler.popleft()
                return budget

            def require(key):
                """Drain filler until `key`'s unit has fully emitted."""
                while key not in done:
                    if not filler:
                        raise AssertionError(f"deadlock: {key} never emitted")
                    pop_ns(220)  # fine steps: stop right at the key

            def drain_filler():
                while filler:
                    pop_ns(10_000)

            MM_NS = 512 * 0.4167

            def proj_qk_unit(dst, wsb, m, n, key):
                def gen():
                    ps = pspool.tile([128, NQ], f32, tag="ps", name="ps")
                    for kc in range(n_kc):
                        nc.tensor.matmul(
                            ps,
                            wsb[:, kc, m * 128:(m + 1) * 128],
                            xt_sb[:, kc, n * NQ:(n + 1) * NQ],
                            start=(kc == 0), stop=(kc == n_kc - 1),
                        )
                        yield MM_NS, None
                    nc.vector.tensor_copy(dst[:, m, n * NQ:(n + 1) * NQ], ps)
                    yield 0, key
                return gen()

            def proj_v_unit(tt):
                def gen():
                    ps = pspool.tile([128, NQ], f32, tag="ps", name="ps")
                    for kc in range(n_kc):
                        nc.tensor.matmul(
                            ps[:, 0:GC],
                            xt_sb[:, kc, tt * 128:(tt + 1) * 128],
                            wv_sb[:, kc, :],
                            start=(kc == 0), stop=(kc == n_kc - 1),
                        )
                        yield MM_NS / 2, None
                    nc.vector.tensor_copy(
                        v_sb[:, tt, :, 0:HD],
                        ps[:, 0:GC].rearrange("p (h d) -> p h d", h=HPC),
                    )
                    yield 0, ("v", tt)
                return gen()

            c2_tiles = {}
            ct_tiles = {}

            def transp_emit(qt, i):
                if qt not in ct_tiles:
                    ct_tiles[qt] = csbpool.tile(
                        [128, 2, NQ], f16, tag="csb", name=f"ct{qt}"
                    )
                ctq = ct_tiles[qt]
                tp = ctpool.tile([128, 4, KB], f32, tag="ct2", name="tp")
                tpv = tp[:, :, :].rearrange("p a b -> p (a b)").bitcast(
                    f16).rearrange("p (a b) -> p a b", a=4)[:, :, 0:KB]
                c2 = c2_tiles[qt]
                for qs in range(4):
                    nc.tensor.transpose(
                        tpv[:, qs, :],
                        c2[:, qs, i * 128:(i + 1) * 128],
                        ident,
                    )
                nc.vector.tensor_copy(ctq[:, i, :], tpv)

            def outproj_half0(qt, m):
                def gen():
                    ctq = ct_tiles[qt]
                    ps = pspool.tile([128, NQ], f32, tag="ps", name="ps")
                    nc.tensor.matmul(
                        ps, wo_sb[:, 0, m * 128:(m + 1) * 128],
                        ctq[:, 0, :], start=True, stop=True,
                    )
                    yield MM_NS, None
                    o_sb = small.tile([128, NQ], f16, tag="o_sb", name="o_sb")
                    nc.vector.tensor_copy(o_sb, ps)
                    nc.sync.dma_start(
                        out=ot2.ap()[m * 128:(m + 1) * 128, :],
                        in_=o_sb,
                    )
                    yield 0, None
                return gen()

            def outproj_half1(qt, m, dst=None):
                def gen():
                    ctq = ct_tiles[qt]
                    ps = (pspool.tile([128, NQ], f32, tag="ps", name="ps")
                          if dst is None else dst)
                    nc.tensor.matmul(
                        ps, wo_sb[:, 1, m * 128:(m + 1) * 128],
                        ctq[:, 1, :], start=True, stop=True,
                    )
                    yield MM_NS, None
                    o_sb = small.tile([128, NQ], f16, tag="o_sb", name="o_sb")
                    if m % 2 == 0:
                        nc.vector.tensor_copy(o_sb, ps)
                    else:
                        nc.scalar.copy(o_sb, ps)
                    nc.sync.dma_start(
                        out=ot.ap()[m * 128:(m + 1) * 128,
                                    qt * NQ:(qt + 1) * NQ],
                        in_=o_sb,
                    )
                    yield 0, None
                return gen()

            def outproj_unit(qt, m):
                def gen():
                    ctq = ct_tiles[qt]
                    ps = pspool.tile([128, NQ], f32, tag="ps", name="ps")
                    for cc in range(2):
                        nc.tensor.matmul(
                            ps,
                            wo_sb[:, cc, m * 128:(m + 1) * 128],
                            ctq[:, cc, :],
                            start=(cc == 0), stop=(cc == 1),
                        )
                        yield MM_NS, None
                    o_sb = small.tile([128, NQ], f16, tag="o_sb", name="o_sb")
                    nc.vector.tensor_copy(o_sb, ps)
                    nc.sync.dma_start(
                        out=ot.ap()[m * 128:(m + 1) * 128,
                                    qt * NQ:(qt + 1) * NQ],
                        in_=o_sb,
                    )
                    yield 0, None
                return gen()

            # ---- attention ----
            finish_prev = [None]

            def attention(h, qt, pop_budget):
                q0 = qt * NQ
                nkb = (q0 + NQ) // KB
                sub, mt = h % 2, h // 2
                r0, r1 = sub * 64, sub * 64 + 64
                ct2 = ctpool.tile([128, 4, KB], f32, tag="ct2", name="ct2")
                pt_prev = None
                first_pv = [True]

                def pv(pt, chunk):
                    for j, kb in chunk:
                        qs_min = max(0, kb - 4 * qt)
                        for qs in range(qs_min, 4):
                            nc.tensor.matmul(
                                ct2[:, qs, 0:HD + 1],
                                pt[:, j, qs * 128:(qs + 1) * 128],
                                v_sb[:, kb, h, :],
                                start=first_pv[0], stop=False,
                                skip_group_check=True,
                            )
                            first_pv[0] = False

                chunks = [
                    [(j, c * 2 + j) for j in range(2)]
                    for c in range(nkb // 2)
                ]
                for c, chunk in enumerate(chunks):
                    where[2] = c
                    require(("v", chunk[-1][1]))
                    st = stpool.tile([128, 2, NQ], f32, tag="st", name="st")
                    off0 = 0
                    for j, kb in chunk:
                        k0 = kb * KB
                        off = max(0, k0 - q0)
                        if j == 0:
                            off0 = off
                        nc.tensor.matmul(
                            st[:, j, off:],
                            kt_sb[r0:r1, mt, k0:k0 + KB],
                            qt_sb[r0:r1, mt, q0 + off:q0 + NQ],
                            start=True, stop=True,
                            tile_position=(r0, 0),
                        )
                    pt = ptpool.tile([128, 2, NQ], f16, tag="pt", name="pt")
                    if c == len(chunks) - 1:
                        # last chunk: per-kb exp so the deferred PV chain
                        # (next head's c0) unblocks sooner
                        for j, kb in chunk:
                            off = max(0, kb * KB - q0)
                            nc.scalar.activation(
                                out=pt[:, j, off:], in_=st[:, j, off:],
                                func=mybir.ActivationFunctionType.Exp,
                                scale=float(1.0 / np.sqrt(HD)),
                            )
                    else:
                        nc.scalar.activation(
                            out=pt[:, :, off0:], in_=st[:, :, off0:],
                            func=mybir.ActivationFunctionType.Exp,
                            scale=float(1.0 / np.sqrt(HD)),
                        )
                    for j, kb in chunk:
                        off = max(0, kb * KB - q0)
                        if kb >= 4 * qt:  # diagonal block: zero k > q
                            nc.vector.tensor_mul(
                                pt[:, j, off:off + KB],
                                pt[:, j, off:off + KB],
                                dmask[:, 0:KB],
                            )
                    if c == 0 and finish_prev[0] is not None:
                        finish_prev[0]()
                        finish_prev[0] = None
                    pop_ns(pop_budget)
                    if pt_prev is not None:
                        pv(*pt_prev)
                    pt_prev = (pt, chunk)

                def finish():
                    pop_ns(PARAMS["final_pop"])
                    pv(*pt_prev)
                    # c2 = ct2[:, :, 0:64] * 1/l, l = ct2[:, :, 64]
                    c2 = c2_tiles[qt]
                    rl = small.tile([128, 4], f32, tag="rl", name="rl")
                    nc.vector.reciprocal(
                        out=rl,
                        in_=ct2[:, :, HD:HD + 1].rearrange(
                            "p a b -> p (a b)"),
                    )
                    for qs in range(4):
                        nc.vector.tensor_scalar(
                            out=c2[:, qs, h * HD:(h + 1) * HD],
                            in0=ct2[:, qs, 0:HD],
                            scalar1=rl[:, qs:qs + 1],
                            scalar2=None,
                            op0=mybir.AluOpType.mult,
                        )
                if h == 0:
                    c2_tiles[qt] = c2pool.tile(
                        [128, 4, GC], f16, tag="c2", name=f"c2_{qt}"
                    )
                finish_prev[0] = finish

            # ---- schedule ----
            where = [0, 0, 0]
            qk_init()
            for m in range(2):
                done.add(("q", m, 0))
                done.add(("k", m, 0))
            for qt in range(n_qt):
                # supply filler for qt+1's attention; m=1 QK lags into
                # qt+1 itself (only needed once its h2 starts)
                n = qt + 1
                if qt == 0:
                    for tt in range(4):
                        filler.append(proj_v_unit(tt))
                if n < n_qt:
                    for dst, wsb, nm in ((qt_sb, wq_sb, "q"),
                                         (kt_sb, wk_sb, "k")):
                        filler.append(proj_qk_unit(dst, wsb, 0, n,
                                                   (nm, 0, n)))
                    for tt in range(4 * n, 4 * n + 4):
                        filler.append(proj_v_unit(tt))
                    for dst, wsb, nm in ((qt_sb, wq_sb, "q"),
                                         (kt_sb, wk_sb, "k")):
                        filler.append(proj_qk_unit(dst, wsb, 1, n,
                                                   (nm, 1, n)))
                if qt > 0:
                    for m in range(n_kc):
                        filler.append(outproj_unit(qt - 1, m))
                budget = PARAMS["budgets"][qt]
                for h in range(HPC):
                    where[0], where[1] = qt, h
                    if h == 2:
                        require(("q", 1, qt))
                        require(("k", 1, qt))
                    if h == 3:
                        transp_emit(qt, 0)
                        if qt == n_qt - 1:
                            for m in range(n_kc):
                                filler.append(outproj_half0(qt, m))
                    attention(h, qt, budget)
                if finish_prev[0] is not None:
                    finish_prev[0]()
                    finish_prev[0] = None
                transp_emit(qt, 1)
                if qt + 1 < n_qt:
                    require(("q", 0, qt + 1))
                    require(("k", 0, qt + 1))
            # final 8 out-proj groups spread across all free psum banks
            sta = stpool.tile([128, 2, NQ], f32, tag="st", name="sta")
            stb = stpool.tile([128, 2, NQ], f32, tag="st", name="stb")
            cta = ctpool.tile([128, 4, KB], f32, tag="ct2", name="cta")
            ctb = ctpool.tile([128, 4, KB], f32, tag="ct2", name="ctb")
            dsts = [None, None, sta[:, 0, :], sta[:, 1, :],
                    stb[:, 0, :], stb[:, 1, :],
                    cta[:, :, :].rearrange("p a b -> p (a b)"),
                    ctb[:, :, :].rearrange("p a b -> p (a b)")]
            for m in range(n_kc):
                filler.append(outproj_half1(n_qt - 1, m, dsts[m]))
            drain_filler()

    nc.compile()
    return nc


def get_nc(t_len=T):
    if t_len not in _cache:
        _cache[t_len] = _build(t_len)
    return _cache[t_len]


def make_in_maps(X, Wq, Wk, Wv, Wo):
    X = np.asarray(X, dtype=np.float32)
    Wq = np.asarray(Wq, dtype=np.float32)
    Wk = np.asarray(Wk, dtype=np.float32)
    Wv = np.asarray(Wv, dtype=np.float32)
    Wo = np.asarray(Wo, dtype=np.float32)
    in_maps = []
    for c in range(NCORES):
        b, g = divmod(c, 4)
        cols = slice(g * GC, (g + 1) * GC)
        in_maps.append({
            "XT": np.ascontiguousarray(X[b].T).astype(np.float16),
            "WQ": np.ascontiguousarray(Wq[:, cols]).astype(np.float16),
            "WK": np.ascontiguousarray(Wk[:, cols]).astype(np.float16),
            "WV": np.ascontiguousarray(Wv[:, cols]).astype(np.float16),
            "WO": np.ascontiguousarray(Wo[cols, :]).astype(np.float16),
        })
    return in_maps


def gather_out(results, bo):
    out = np.zeros((B, T, D), dtype=np.float32)
    for c in range(NCORES):
        out[c // 4] += results[c]["OT"].T.astype(np.float32)
        out[c // 4][T - NQ:] += results[c]["OT2"].T.astype(np.float32)
    out += np.asarray(bo, dtype=np.float32)
    return out


def kernel(X, Wq, Wk, Wv, Wo, bo):
    from concourse import bass_utils

    nc = get_nc(T)
    in_maps = make_in_maps(X, Wq, Wk, Wv, Wo)
    res = bass_utils.run_bass_kernel_spmd(
        nc, in_maps, core_ids=list(range(NCORES))
    )
    return gather_out(res.results, bo)
